# revision 2
# baseline (speedup 1.0000x reference)
"""Trainium2 Bass kernel for nn_AttentionLayer (B=16, S=2048, D=512, H=64).

Data-parallel over batch: 8 NeuronCores x 2 batch items each; no collectives.

Math (per batch item b):
  q = x @ Wq + bq;  k = x @ Wk          [S, H]   (bk provably cancels in the
  scores = q @ k.T / sqrt(H)            [S, S]    row-softmax; bq folds into q)
  w = softmax(scores, axis=-1)          (no rowmax pass: |scores| <= ~9)
  cbar[t] = (1/S) sum_s w[s,t]
  out = cbar @ v + bv,  v = x @ Wv      [H]

Engine assignment (critical path = ScalarE exp ~ DVE):
  - ScalarE: exact exp (activation w/ accum_out -> Z) on 12 of 16 tiles/batch
  - DVE: Schraudolph approx exp (i32(x*2^23/ln2 + B), bitcast f32; colsum
    reads the odd-u16 halves as bf16) on DVE_TILES, their Z row-sums, and
    all PSUM evacuations + reciprocals
  - TensorE: projections, row-packed scores (K=64 at (0,0)/(64,0)),
    col-packed colsum, v tiles, tail transposes
  - DMA: f32->bf16 cast DRAM->DRAM (SWDGE) then xbar transposes DRAM->SBUF,
    serialized cast->transposes->cast per half-batch (xbar transposes must
    not overlap other DMA traffic - measured data corruption otherwise)

Emission schedule: colsums delayed one pair (they wait on rz and would
block the in-order PE queue); v-tile groups sprinkled between pairs;
batch-1 projections emitted before batch-0's trailing colsums; tail does
cbar^T via 16 tiny PE transposes then 16 N=64 matmuls against v.
"""

import numpy as np

B, S, D, H = 16, 2048, 512, 64
NCORES = 8
BPC = B // NCORES  # batches per core
P = 128
NT = S // P  # 16 row tiles
ND = D // P  # 4 d tiles
NC4 = S // 512  # 4 free-dim chunks of 512

# row tiles whose exp runs on DVE (Schraudolph); rest on ScalarE
DVE_TILES = (3, 7, 11, 15)

SCH_A = float(2.0**23 / np.log(2.0))
SCH_B = float(127 * 2**23 - 366393)


def build_nc():
    import concourse.bacc as bacc
    import concourse.mybir as mybir
    import concourse.tile as tile
    from concourse.tile_rust import add_dep_helper

    f32 = mybir.dt.float32
    bf16 = mybir.dt.bfloat16
    i32 = mybir.dt.int32
    u16 = mybir.dt.uint16
    Exp = mybir.ActivationFunctionType.Exp
    X = mybir.AxisListType.X
    add = mybir.AluOpType.add
    mult = mybir.AluOpType.mult

    nc = bacc.Bacc("TRN2", target_bir_lowering=False)

    x_ext = nc.declare_dram_parameter("inputs", [BPC, S, D], f32, isOutput=False)
    wq_ext = nc.declare_dram_parameter("Wq", [D, H], f32, isOutput=False)
    bq_ext = nc.declare_dram_parameter("bq", [H], f32, isOutput=False)
    wk_ext = nc.declare_dram_parameter("Wk", [D, H], f32, isOutput=False)
    bk_ext = nc.declare_dram_parameter("bk", [H], f32, isOutput=False)  # noqa: F841
    wv_ext = nc.declare_dram_parameter("Wv", [D, H], f32, isOutput=False)
    bv_ext = nc.declare_dram_parameter("bv", [H], f32, isOutput=False)
    out_ext = nc.declare_dram_parameter("out", [BPC, H], f32, isOutput=True)

    inv_sqrt_h = 1.0 / float(np.sqrt(H))

    with tile.TileContext(nc) as tc:
        with (
            tc.tile_pool(name="singles", bufs=1) as singles,
            tc.tile_pool(name="xT", bufs=2) as xT_pool,
            tc.tile_pool(name="qkT", bufs=4) as qkT_pool,
            tc.tile_pool(name="w", bufs=5) as w_pool,
            tc.tile_pool(name="ei", bufs=3) as ei_pool,
            tc.tile_pool(name="v", bufs=2) as v_pool,
            tc.tile_pool(name="zr", bufs=10) as zr_pool,
            tc.tile_pool(name="misc", bufs=4) as misc_pool,
            tc.tile_pool(name="dram", bufs=2, space="DRAM") as dram_pool,
            tc.tile_pool(name="mm", bufs=3, space="PSUM") as mm_pool,
            tc.tile_pool(name="col", bufs=2, space="PSUM") as col_pool,
        ):
            # ---- weights / biases prep (once) ----
            wq_f = singles.tile([P, ND, H], f32)
            nc.sync.dma_start(out=wq_f, in_=wq_ext.rearrange("(j p) h -> p j h", p=P))
            wk_f = singles.tile([P, ND, H], f32)
            nc.sync.dma_start(out=wk_f, in_=wk_ext.rearrange("(j p) h -> p j h", p=P))
            wv_f = singles.tile([P, ND, H], f32)
            nc.sync.dma_start(out=wv_f, in_=wv_ext.rearrange("(j p) h -> p j h", p=P))

            # Duplicated projection stationaries: wq2 = [Wq/sqrt(H) | Wq/sqrt(H)],
            # wk2 = [Wk | Wk] -> psum outs come out as [qT;qT] / [kT;kT].
            wq2 = singles.tile([P, ND, P], bf16)
            wk2 = singles.tile([P, ND, P], bf16)
            for j in range(ND):
                for hhalf in range(2):
                    sl = slice(hhalf * H, (hhalf + 1) * H)
                    nc.vector.tensor_scalar(
                        out=wq2[:, j, sl], in0=wq_f[:, j, :],
                        scalar1=inv_sqrt_h, scalar2=None, op0=mult,
                    )
                    nc.vector.tensor_copy(out=wk2[:, j, sl], in_=wk_f[:, j, :])
            wv_b = singles.tile([P, ND, H], bf16)
            for j in range(ND):
                nc.vector.tensor_copy(out=wv_b[:, j, :], in_=wv_f[:, j, :])

            # bq/sqrt(H), duplicated per half (bk cancels in softmax)
            bias_q2 = singles.tile([P, 1], f32)
            for hhalf in range(2):
                sl = slice(hhalf * H, (hhalf + 1) * H)
                nc.sync.dma_start(out=bias_q2[sl, 0:1], in_=bq_ext[:, None])
            nc.vector.tensor_scalar(
                out=bias_q2, in0=bias_q2, scalar1=inv_sqrt_h, scalar2=None, op0=mult,
            )
            bv_sb = singles.tile([1, H], f32)
            nc.sync.dma_start(out=bv_sb, in_=bv_ext[None, :])
            ones4 = singles.tile([P, 1], bf16)
            nc.vector.memset(ones4, 1.0)

            # ---- input pipeline ----
            # Serial per half-batch groups: cast -> transposes -> next cast
            # (xbar transposes must not overlap other DMA traffic)
            xT_tiles = []
            prev_gate = None
            for b in range(BPC):
                xs = dram_pool.tile([S, D], bf16, tag="xs")
                xT = xT_pool.tile([P, ND, S], bf16, tag="xT")
                for rh in range(2):
                    rsl = slice(rh * 1024, (rh + 1) * 1024)
                    ci = nc.gpsimd.dma_start(out=xs[rsl, :], in_=x_ext[b][rsl, :])
                    if prev_gate is not None:
                        add_dep_helper(
                            ci.ins, prev_gate,
                            reason="hold cast until prior group transposes drain",
                        )
                    ti = None
                    for j in range(ND):
                        ti = nc.sync.dma_start_transpose(
                            out=xT[:, j, rsl],
                            in_=xs[rsl, j * P : (j + 1) * P],
                        )
                    prev_gate = ti.ins
                xT_tiles.append(xT)

            # ---- per-batch emission helpers ----
            state = {}

            def proj(b):
                xT = xT_tiles[b]
                qT2 = qkT_pool.tile([P, S], bf16, tag="qT2", name=f"qT2_{b}")
                kT2 = qkT_pool.tile([P, S], bf16, tag="kT2", name=f"kT2_{b}")
                for c in range(NC4):
                    cs = slice(c * 512, (c + 1) * 512)
                    pp = mm_pool.tile([P, 1024], f32, tag="mm", name=f"proj_{b}_{c}")
                    for j in range(ND):
                        nc.tensor.matmul(
                            pp[:, 0:512], lhsT=wq2[:, j, :], rhs=xT[:, j, cs],
                            start=(j == 0), stop=(j == ND - 1),
                        )
                    for j in range(ND):
                        nc.tensor.matmul(
                            pp[:, 512:1024], lhsT=wk2[:, j, :], rhs=xT[:, j, cs],
                            start=(j == 0), stop=(j == ND - 1),
                        )
                    nc.vector.tensor_scalar(
                        out=qT2[:, cs], in0=pp[:, 0:512],
                        scalar1=bias_q2[:, 0:1], scalar2=None, op0=add,
                    )
                    nc.vector.tensor_copy(out=kT2[:, cs], in_=pp[:, 512:1024])
                colbank = col_pool.tile([P, 512], f32, tag="col", name=f"colbank_{b}")
                state[b] = {
                    "qT2": qT2, "kT2": kT2, "colbank": colbank,
                    "wsrc": [None] * NT, "rzb": [None] * NT,
                }

            def pair(b, p_i):
                st = state[b]
                qT2, kT2 = st["qT2"], st["kT2"]
                i0, i1 = 2 * p_i, 2 * p_i + 1
                outs = {}
                for i in (i0, i1):
                    if i in DVE_TILES:
                        ei = ei_pool.tile([P, S], i32, tag="ei", name=f"ei_{b}_{i}")
                        outs[i] = ("dve", ei)
                        eu = ei.bitcast(u16).rearrange(
                            "p (d two) -> p d two", two=2
                        )[:, :, 1]
                        st["wsrc"][i] = eu.bitcast(bf16)
                    else:
                        w_t = w_pool.tile([P, S], bf16, tag="w", name=f"w_{b}_{i}")
                        outs[i] = ("sc", w_t)
                        st["wsrc"][i] = w_t

                z0 = zr_pool.tile([P, 2], f32, tag="z", name=f"z_{b}_{i0}")
                z1 = zr_pool.tile([P, 2], f32, tag="z", name=f"z_{b}_{i1}")
                for hhalf in range(2):
                    psa = mm_pool.tile(
                        [P, 1024], f32, tag="mm", name=f"psa_{b}_{p_i}_{hhalf}"
                    )
                    psb = mm_pool.tile(
                        [P, 1024], f32, tag="mm", name=f"psb_{b}_{p_i}_{hhalf}"
                    )
                    for c2 in range(2):
                        t0 = hhalf * 1024 + c2 * 512
                        csl = slice(c2 * 512, (c2 + 1) * 512)
                        nc.tensor.matmul(
                            psa[:, csl],
                            lhsT=qT2[0:H, i0 * P : (i0 + 1) * P],
                            rhs=kT2[0:H, t0 : t0 + 512],
                            start=True, stop=True, tile_position=(0, 0),
                        )
                        nc.tensor.matmul(
                            psb[:, csl],
                            lhsT=qT2[H:P, i1 * P : (i1 + 1) * P],
                            rhs=kT2[H:P, t0 : t0 + 512],
                            start=True, stop=True, tile_position=(H, 0),
                        )
                    hs = slice(hhalf * 1024, (hhalf + 1) * 1024)
                    for i, ps, zt in ((i0, psa, z0), (i1, psb, z1)):
                        kind, dst = outs[i]
                        if kind == "sc":
                            nc.scalar.activation(
                                out=dst[:, hs], in_=ps[:], func=Exp,
                                accum_out=zt[:, hhalf : hhalf + 1],
                            )
                        else:
                            nc.vector.tensor_scalar(
                                out=dst[:, hs], in0=ps[:],
                                scalar1=SCH_A, scalar2=SCH_B,
                                op0=mult, op1=add,
                            )

                for i, zt in ((i0, z0), (i1, z1)):
                    kind, dst = outs[i]
                    zs = zr_pool.tile([P, 1], f32, tag="zs", name=f"zs_{b}_{i}")
                    if kind == "sc":
                        nc.vector.tensor_add(out=zs, in0=zt[:, 0:1], in1=zt[:, 1:2])
                    else:
                        nc.vector.reduce_sum(out=zs, in_=dst.bitcast(f32), axis=X)
                    rzf = zr_pool.tile([P, 1], f32, tag="rzf", name=f"rzf_{b}_{i}")
                    nc.vector.reciprocal(out=rzf, in_=zs)
                    rzb = zr_pool.tile([P, 1], bf16, tag="rzb", name=f"rzb_{b}_{i}")
                    # fold the 1/S mean-pool into the stationary
                    nc.vector.tensor_scalar(
                        out=rzb, in0=rzf, scalar1=1.0 / float(S),
                        scalar2=None, op0=mult,
                    )
                    st["rzb"][i] = rzb

            def colsum(b, i):
                st = state[b]
                for c in range(NC4):
                    nc.tensor.matmul(
                        st["colbank"][32 * c : 32 * c + 1, :],
                        lhsT=st["rzb"][i],
                        rhs=st["wsrc"][i][:, c * 512 : (c + 1) * 512],
                        start=(i == 0), stop=(i == NT - 1),
                        tile_position=(0, 32 * c),
                    )

            def vgrp(b, g):
                # v tiles 4g..4g+3, natural layout, packed into one psum alloc
                xT = xT_tiles[b]
                if g == 0:
                    state[b]["v"] = v_pool.tile(
                        [P, NT, H], bf16, tag="v", name=f"v_{b}"
                    )
                vp = mm_pool.tile([P, 1024], f32, tag="mm", name=f"vp_{b}_{g}")
                for tt in range(4):
                    t = 4 * g + tt
                    vsl = slice(tt * H, (tt + 1) * H)
                    for j in range(ND):
                        nc.tensor.matmul(
                            vp[:, vsl],
                            lhsT=xT[:, j, t * P : (t + 1) * P],
                            rhs=wv_b[:, j, :],
                            start=(j == 0), stop=(j == ND - 1),
                        )
                nc.vector.tensor_copy(
                    out=state[b]["v"][:, 4 * g : 4 * g + 4, :].rearrange(
                        "p t h -> p (t h)"
                    ),
                    in_=vp[:, 0:256],
                )

            def tail(b):
                st = state[b]
                cbar_bf = misc_pool.tile([P, 512], bf16, tag="cbar", name=f"cb_{b}")
                nc.vector.tensor_copy(out=cbar_bf, in_=st["colbank"])
                tp_t = col_pool.tile([P, 2 * NT], bf16, tag="col", name=f"tailt_{b}")
                for c in range(NC4):
                    for i in range(4):
                        t = 4 * c + i
                        nc.tensor.transpose(
                            out=tp_t[:, 2 * t : 2 * t + 1],
                            in_=cbar_bf[32 * c : 32 * c + 1, i * P : (i + 1) * P],
                            identity=ones4[32 * c : 32 * c + 1, 0:1],
                            tile_position=(32 * c, 0),
                        )
                cbT = misc_pool.tile([P, NT], bf16, tag="cbT", name=f"cbT_{b}")
                nc.vector.tensor_copy(
                    out=cbT,
                    in_=tp_t.rearrange("p (t two) -> p t two", two=2)[:, :, 0],
                )
                tp_o = mm_pool.tile([P, 1024], f32, tag="mm", name=f"tailo_{b}")
                for t in range(NT):
                    nc.tensor.matmul(
                        tp_o[0:1, 0:H],
                        lhsT=cbT[:, t : t + 1],
                        rhs=st["v"][:, t, :],
                        start=(t == 0), stop=(t == NT - 1),
                    )
                o_sb = misc_pool.tile([1, H], f32, tag="o", name=f"o_{b}")
                nc.vector.tensor_add(out=o_sb, in0=tp_o[0:1, 0:H], in1=bv_sb)
                nc.sync.dma_start(out=out_ext[b : b + 1, :], in_=o_sb)

            # ---- schedule ----
            def emit_pairs(b, first_pair):
                for p_i in range(first_pair, NT // 2):
                    pair(b, p_i)
                    if p_i >= 1:
                        colsum(b, 2 * p_i - 2)
                        colsum(b, 2 * p_i - 1)
                    if p_i in (2, 4, 6):
                        vgrp(b, p_i // 2 - 1)

            proj(0)
            emit_pairs(0, 0)
            proj(1)
            colsum(0, NT - 2)
            colsum(0, NT - 1)
            vgrp(0, 3)
            pair(1, 0)
            tail(0)
            emit_pairs(1, 1)
            colsum(1, NT - 2)
            colsum(1, NT - 1)
            vgrp(1, 3)
            tail(1)

    nc.finalize()
    return nc


_NC_CACHE = None


def _get_nc():
    global _NC_CACHE
    if _NC_CACHE is None:
        _NC_CACHE = build_nc()
    return _NC_CACHE


def run(inputs_map, trace=False, **spmd_kwargs):
    from concourse.bass_utils import run_bass_kernel_spmd

    x = np.ascontiguousarray(np.asarray(inputs_map["inputs"], dtype=np.float32))
    assert x.shape == (B, S, D), x.shape
    full = {
        "Wq": np.ascontiguousarray(np.asarray(inputs_map["Wq"], np.float32)),
        "bq": np.ascontiguousarray(np.asarray(inputs_map["bq"], np.float32)),
        "Wk": np.ascontiguousarray(np.asarray(inputs_map["Wk"], np.float32)),
        "bk": np.ascontiguousarray(np.asarray(inputs_map["bk"], np.float32)),
        "Wv": np.ascontiguousarray(np.asarray(inputs_map["Wv"], np.float32)),
        "bv": np.ascontiguousarray(np.asarray(inputs_map["bv"], np.float32)),
    }
    in_maps = []
    for i in range(NCORES):
        m = {"inputs": np.ascontiguousarray(x[i * BPC : (i + 1) * BPC])}
        m.update(full)
        in_maps.append(m)
    nc = _get_nc()
    res = run_bass_kernel_spmd(
        nc, in_maps, core_ids=list(range(NCORES)), trace=trace, **spmd_kwargs
    )
    out = np.concatenate([np.asarray(res.results[i]["out"]) for i in range(NCORES)], 0)
    return out.astype(np.float32), res


def kernel(**inputs):
    out, _ = run(inputs, trace=False)
    return out


if __name__ == "__main__":
    rng = np.random.default_rng(0)
    ins = {
        "inputs": rng.standard_normal((B, S, D), dtype=np.float32),
        "Wq": rng.standard_normal((D, H), dtype=np.float32) / np.sqrt(D),
        "bq": np.zeros(H, np.float32),
        "Wk": rng.standard_normal((D, H), dtype=np.float32) / np.sqrt(D),
        "bk": np.zeros(H, np.float32),
        "Wv": rng.standard_normal((D, H), dtype=np.float32) / np.sqrt(D),
        "bv": np.zeros(H, np.float32),
    }
    out = kernel(**ins)
    print("out", out.shape, out[0, :4])
